# revision 34
# baseline (speedup 1.0000x reference)
"""Trainium2 Bass kernel for nn_AttentionLayer (B=8, N=2048, D=512).

Sharding: data-parallel over batch — one batch element per NeuronCore (8 cores),
no collectives.

Per-core pipeline (x_b [2048, 512]), chunk-major over 4 q-chunks of 512 rows so
LN/expand/attention/project of successive chunks overlap:
  1. LayerNorm in natural layout; PE-transpose nx -> nx_T (per-chunk tiles).
  2. Expand GEMM (h = nx @ expand, 2176 cols; columns host-reordered to
     [q | k | local-lin | local-gelu | v-lin | v-gelu]) split by consumer:
       - q/k/local computed TRANSPOSED (h_T = expand.T @ nx)
       - v computed NATURAL (rows on partitions)
     so attention needs no score/v transposes.
  3. Logits transposed [k, q] directly from q_T/k_T. The sigmoid recency
     bias sigmoid((k-q)+4) underflows to 0 for q-k >= 129, so interior
     k-tiles need no mask at all (e = Exp(lg)); only the 5 band patterns
     (sub-diagonal + 4 diagonal, identical across chunks) use a
     host-precomputed fp16 mask, resident in SBUF. Diagonal tiles also skip
     their fully-masked leading q-columns in logits/e/attn (causal slicing).
  4. attn_T = v.T @ e (unnormalized, 4 PSUM banks). Softmax denominator is
     accumulated as an elementwise e-sum on DVE (GpSimd also works but showed
     rare cross-engine write-visibility flakes), reduced with one ones-matmul
     per chunk, transposed to per-partition [128,4] via PE contraction-1
     matmuls, reciprocal'd (one Newton step); normalization is applied after
     the project GEMM (per-partition DVE scale).
  5. Project GEMM consumes local_T/attn_T as stationary operands; residual
     added on DVE. Causal structure skips fully-masked k-tiles (40/64 kept).

DMA strategy: HWDGE issue costs ~625ns per DMA regardless of size, so DMAs are
merged: expand weights 8 DMAs (resident all chunks, T-part columns first for
startup), project 1, x 1/chunk (4 for chunk 0), y 1/chunk, diagonal masks 1.
All matmuls in float32r (full PE rate at free-dim >= 256).
"""

import numpy as np

import concourse.bass as bass
import concourse.mybir as mybir
import concourse.tile as tile
import concourse.bass_utils as bass_utils
from concourse.masks import make_identity
from concourse import bass_isa
from concourse.vector_clock import ScopedClock

F32 = mybir.dt.float32
F32R = mybir.dt.float32r
F16 = mybir.dt.float16
AF = mybir.ActivationFunctionType
ALU = mybir.AluOpType
X_AX = mybir.AxisListType.X

B = 8
N = 2048
D = 512
QK = 64
ED = 1024
OUTE = 2176
LN_EPS = 1e-5
NT = N // 128      # 16 row tiles
KT = D // 128      # 4 contraction tiles (feature dim)
NCH = 4            # q chunks of 512
CH = N // NCH      # 512

# column layout of the host-reordered expand matrix
C_Q = 0            # [0, 64)      q
C_K = 64           # [64, 128)    k
C_LL = 128         # [128, 640)   local linear
C_LG = 640         # [640, 1152)  local pre-gelu
C_VL = 1152        # [1152, 1664) v linear
C_VG = 1664        # [1664, 2176) v pre-gelu


# ----------------------------------------------------------------------------
# Workaround for the walrus build in this container: CTRL-class instructions
# (Drain/NoOp) support only ONE sync-wait command. Split multi-wait
# instructions by hoisting extra waits onto preceding same-engine NOPs.
# ----------------------------------------------------------------------------
_SPLIT_LIMIT = 1
_patched = [False]


def _apply_patches():
    if _patched[0]:
        return
    _patched[0] = True

    orig_add = tile.TileContext._add_instruction
    ctr = [0]

    def _split_add(self, inst):
        si = inst.sync_info
        if (si is not None and si.on_wait and len(si.on_wait) > _SPLIT_LIMIT
                and inst.engine != mybir.EngineType.Unassigned):
            waits = list(si.on_wait)
            for w in waits[:-_SPLIT_LIMIT]:
                ctr[0] += 1
                nop = mybir.InstNoOp(name=f"I-waitsplit-{ctr[0]}", ins=[], outs=[])
                nop.engine = inst.engine
                nop.sync_info = mybir.SyncInfo(on_wait=[w], on_update=[])
                orig_add(self, nop)
            si.on_wait = waits[-_SPLIT_LIMIT:]
        orig_add(self, inst)

    tile.TileContext._add_instruction = _split_add

    def _patched_drain_and_barrier(self, tick_clock, wait_clock):
        nc = self.nc
        drain_inst = nc.sync.drain()
        wait_clock.add_sem_waits(
            drain_inst.ins, ScopedClock({None: tick_clock.global_clock})
        )
        si = drain_inst.ins.sync_info
        if si is not None and si.on_wait and len(si.on_wait) > _SPLIT_LIMIT:
            waits = list(si.on_wait)
            si.on_wait = waits[:_SPLIT_LIMIT]
            for w in waits[_SPLIT_LIMIT:]:
                d2 = nc.sync.drain()
                s2 = d2.ins.sync_info
                if s2 is None:
                    d2.ins.sync_info = mybir.SyncInfo(on_wait=[w], on_update=[])
                else:
                    s2.on_wait = [w]
        nc.all_engine_barrier()
        popped = nc._tile_sem_poison_stack.pop()
        assert popped is self._sem_poison
        nc.clear_and_free_semaphores(list(self.sems.allocated().values()))
        nc.all_engine_barrier()

    tile.TileContext._drain_and_barrier = _patched_drain_and_barrier


def _emit(nc, tc):
    x = nc.dram_tensor("x", [N, D], F32, kind="ExternalInput").ap()
    expd = nc.dram_tensor("expand", [D, OUTE], F32, kind="ExternalInput").ap()
    projd = nc.dram_tensor("project", [ED, D], F32, kind="ExternalInput").ap()
    # 5 band mask tile patterns (identical across chunks), [128, 5*512]:
    # band -1 (sub-diagonal) then bands 0..3 (diagonal)
    maskd = nc.dram_tensor("maskdiag", [128, 5 * CH], F16,
                           kind="ExternalInput").ap()
    y = nc.dram_tensor("y", [N, D], F32, kind="ExternalOutput").ap()

    from contextlib import ExitStack
    with ExitStack() as _ctx:
        def _pool(name, bufs, space="SBUF"):
            return _ctx.enter_context(
                tc.tile_pool(name=name, bufs=bufs, space=space))

        constp = _pool("constp", 1)
        pp = _pool("pp", 1)
        wp = _pool("wp", 1)
        xpp = _pool("xpp", 2)
        nxTp = _pool("nxTp", 8)
        qp = _pool("qp", 2)
        gltp = _pool("gltp", 2)
        ep = _pool("ep", 4)
        lmp = _pool("lmp", 2)
        asbp = _pool("asbp", 1)
        misc = _pool("misc", 2)
        stp = _pool("stp", 4)
        outp = _pool("outp", 2)
        denp = _pool("denp", 2)
        ps1 = _pool("ps1", 2, space="PSUM")
        psL = _pool("psL", 2, space="PSUM")
        psB = _pool("psB", 1, space="PSUM")

        ident = constp.tile([128, 128], F32, tag="ident")
        make_identity(nc, ident)
        ones_f = constp.tile([128, 1], F32, tag="ones_f")
        nc.vector.memset(ones_f, 1.0)
        ones = constp.tile([128, 1], F32R, tag="ones")
        nc.vector.tensor_copy(ones, ones_f)
        epst = constp.tile([128, 1], F32, tag="epst")
        nc.vector.memset(epst, LN_EPS)
        # Square scale 1/sqrt(D): accum_out then sums (x/sqrt(D))^2 = ssq/D
        sclD = constp.tile([128, 1], F32, tag="sclD")
        nc.vector.memset(sclD, float(1.0 / np.sqrt(D)))

        # persistent across chunks
        k_all = [pp.tile([64, CH], F32R, tag=f"k{c}", name=f"k{c}")
                 for c in range(NCH)]
        v_sb = [pp.tile([128, D], F32R, tag=f"v{r}", name=f"v{r}")
                for r in range(NT)]

        def x_dma(c):
            xc = xpp.tile([128, NCH * D], F32, tag="x", name=f"x_{c}")
            if c == 0:
                # split per row-tile so LN of t=0 starts ~2us earlier
                for t in range(4):
                    nc.sync.dma_start(
                        xc[:, t * D:(t + 1) * D],
                        x[t * 128:(t + 1) * 128, :])
            else:
                nc.sync.dma_start(
                    xc.rearrange("p (t d) -> p t d", t=NCH),
                    x[c * CH:(c + 1) * CH, :].rearrange("(t p) d -> p t d",
                                                        p=128))
            return xc

        def ln_chunk(c, xc):
            nxT = [nxTp.tile([128, CH], F32R, tag="nxT", name=f"nxT{c}_{kt}")
                   for kt in range(KT)]
            for t in range(4):
                xt = xc[:, t * D:(t + 1) * D]
                mu = stp.tile([128, 1], F32, tag="mu")
                nc.vector.reduce_sum(out=mu, in_=xt, axis=X_AX)
                sq = misc.tile([128, D], F32, tag="sq", bufs=1)
                ssq = stp.tile([128, 1], F32, tag="ssq")
                nc.scalar.activation(sq, xt, AF.Square, scale=sclD,
                                     accum_out=ssq)
                nc.vector.tensor_scalar_mul(mu, mu, 1.0 / D)
                musq = stp.tile([128, 1], F32, tag="musq")
                nc.vector.tensor_mul(musq, mu, mu)
                var = stp.tile([128, 1], F32, tag="var")
                nc.vector.tensor_sub(var, ssq, musq)
                std = stp.tile([128, 1], F32, tag="std")
                nc.scalar.activation(std, var, AF.Sqrt, bias=epst)
                rstd = stp.tile([128, 1], F32, tag="rstd")
                nc.vector.reciprocal(rstd, std)
                nxt = misc.tile([128, D], F32, tag="nx")
                nc.vector.tensor_scalar(nxt, xt, mu, rstd,
                                        op0=ALU.subtract, op1=ALU.mult)
                tp = ps1.tile([128, 512], F32, tag="ps")
                for j in range(KT):
                    nc.tensor.matmul(tp[:, j * 128:(j + 1) * 128],
                                     nxt[:, j * 128:(j + 1) * 128], ident,
                                     is_transpose=True, skip_group_check=True)
                for j in range(KT):
                    nc.scalar.copy(nxT[j][:, t * 128:(t + 1) * 128],
                                   tp[:, j * 128:(j + 1) * 128])
            return nxT

        xc0 = x_dma(0)

        # resident weights: emitted after chunk-0 x DMA so LN starts first.
        # T-part columns [0, 1152) land first (needed ~T+8us), the natural
        # v-columns [1152, 2176) and project/mask after.
        expsb = []
        for kt in range(KT):
            w = wp.tile([128, OUTE], F32R, tag=f"exp{kt}", name=f"exp{kt}")
            nc.sync.dma_start(
                w[:, 0:C_VL],
                expd[kt * 128:(kt + 1) * 128, 0:C_VL].bitcast(F32R))
            expsb.append(w)
        for kt in range(KT):
            nc.sync.dma_start(
                expsb[kt][:, C_VL:OUTE],
                expd[kt * 128:(kt + 1) * 128, C_VL:OUTE].bitcast(F32R))
        projsb = wp.tile([128, 8 * 512], F32R, tag="projsb")
        nc.sync.dma_start(
            projsb.rearrange("p (j d) -> p j d", j=8),
            projd.rearrange("(j p) d -> p j d", p=128).bitcast(F32R))
        maskt = wp.tile([128, 5 * CH], F16, tag="maskt")
        nc.sync.dma_start(maskt, maskd)

        pend = ln_chunk(0, xc0)

        for c in range(NCH):
            nxT = pend
            xc = xc0
            # ---------------- expand T-part for this chunk -----------------
            def t_mm(ps_ap, c0, m):
                for kt in range(KT):
                    nc.tensor.matmul(ps_ap[:m, :],
                                     expsb[kt][:, c0:c0 + m], nxT[kt],
                                     start=(kt == 0), stop=(kt == KT - 1))

            q_sb = qp.tile([64, CH], F32R, tag="q_sb", name=f"q_{c}")
            ps = psB.tile([128, 512], F32, tag="a2")
            t_mm(ps, C_Q, 64)
            nc.scalar.copy(q_sb, ps[:64, :])
            ps = psB.tile([128, 512], F32, tag="a3")
            t_mm(ps, C_K, 64)
            nc.scalar.copy(k_all[c], ps[:64, :])

            glt = []
            for j in range(4):
                if j % 2 == 0:
                    psl = psB.tile([128, 512], F32, tag="a2")
                    psg = psB.tile([128, 512], F32, tag="a3")
                else:
                    psl = psL.tile([128, 512], F32, tag="lg")
                    psg = psL.tile([128, 512], F32, tag="lg")
                t_mm(psl, C_LL + 128 * j, 128)
                t_mm(psg, C_LG + 128 * j, 128)
                gelt = misc.tile([128, 512], F32, tag="gelt")
                nc.scalar.activation(gelt, psg, AF.Gelu)
                g = gltp.tile([128, CH], F32R, tag=f"glt{j}", name=f"glt{j}_{c}")
                nc.vector.tensor_mul(g, psl, gelt)
                glt.append(g)

            # ---------------- expand natural part -> v for this chunk ------
            for t in range(4):
                r = 4 * c + t
                if t % 2 == 0:
                    pl = psB.tile([128, 512], F32, tag="a2")
                    pg = psB.tile([128, 512], F32, tag="a3")
                else:
                    pl = psL.tile([128, 512], F32, tag="lg")
                    pg = psL.tile([128, 512], F32, tag="lg")
                for kt in range(KT):
                    nc.tensor.matmul(pl, nxT[kt][:, t * 128:(t + 1) * 128],
                                     expsb[kt][:, C_VL:C_VL + 512],
                                     start=(kt == 0), stop=(kt == KT - 1))
                for kt in range(KT):
                    nc.tensor.matmul(pg, nxT[kt][:, t * 128:(t + 1) * 128],
                                     expsb[kt][:, C_VG:C_VG + 512],
                                     start=(kt == 0), stop=(kt == KT - 1))
                vg = misc.tile([128, 512], F32, tag="vg")
                nc.scalar.activation(vg, pg, AF.Gelu)
                nc.vector.tensor_mul(v_sb[r], pl, vg)

            if c + 1 < NCH:
                xc0 = x_dma(c + 1)

            # ---------------- attention for this chunk ---------------------
            oc = outp.tile([128, NCH * D], F32, tag="oc", name=f"oc_{c}")
            nkt = 4 * c + 4
            attn_ps = [psB.tile([128, 512], F32, tag=f"a{j}", name=f"a{j}_{c}")
                       for j in range(4)]
            dacc = denp.tile([128, 512], F32R, tag="dacc", name=f"dacc_{c}",
                             bufs=1)
            e_last = None
            e_last_lo = 0
            for kt in range(nkt):
                band = kt - 4 * c
                # diagonal band t: q-columns below t*128 are fully masked
                lo = band * 128 if band > 0 else 0
                # 3-deep logits bank rotation: ps1 is idle during
                # attention (next LN is emitted after asb; chunk 3's early
                # o1 shares the ps1 queue safely), so borrow a bank
                if kt % 3 == 2:
                    lg = ps1.tile([128, 512], F32, tag="ps")
                else:
                    lg = psL.tile([128, 512], F32, tag="lg")
                nc.tensor.matmul(lg[:, lo:],
                                 k_all[kt // 4][:, (kt % 4) * 128:
                                                (kt % 4 + 1) * 128],
                                 q_sb[:, lo:])
                e = ep.tile([128, 512], F32R, tag="e", name=f"e_{c}_{kt}")
                if band >= -1:
                    # sub-diagonal and diagonal bands carry the sigmoid
                    # recency bias / causal mask
                    mi = band + 1
                    lm = lmp.tile([128, 512], F32, tag="lm")
                    nc.vector.tensor_add(lm[:, lo:], lg[:, lo:],
                                         maskt[:, mi * CH + lo:(mi + 1) * CH])
                    nc.scalar.activation(e[:, lo:], lm[:, lo:], AF.Exp)
                else:
                    # interior: sigmoid(rel+4) < 1e-54 -> e = exp(lg)
                    nc.scalar.activation(e, lg, AF.Exp)
                for j in range(4):
                    nc.tensor.matmul(attn_ps[j][:, lo:],
                                     v_sb[kt][:, j * 128:(j + 1) * 128],
                                     e[:, lo:],
                                     start=(kt == 0), stop=(kt == nkt - 1),
                                     skip_group_check=True)
                if c == NCH - 1 and kt >= 8 and kt % 2 == 0:
                    t_e = (kt - 8) // 2
                    o1e = ps1.tile([128, 512], F32, tag="ps")
                    for j in range(4):
                        nc.tensor.matmul(
                            o1e, glt[j][:, t_e * 128:(t_e + 1) * 128],
                            projsb[:, j * 512:(j + 1) * 512],
                            start=(j == 0), stop=(j == 3))
                    nc.vector.tensor_add(oc[:, t_e * D:(t_e + 1) * D], o1e,
                                         xc[:, t_e * D:(t_e + 1) * D])
                # softmax denominator partial sums on the idle Pool engine
                # (the last tile goes straight into the ones-matmul below)
                if kt == 0:
                    nc.vector.tensor_copy(dacc, e)
                elif kt < nkt - 1:
                    nc.vector.tensor_add(dacc[:, lo:], dacc[:, lo:],
                                         e[:, lo:])
                else:
                    e_last, e_last_lo = e, lo

            # denominator: ones-matmul -> [1,512]; reciprocal; SBUF->SBUF DMA
            # transpose to per-partition [128, 4]
            den_ps = psL.tile([1, 512], F32, tag="lg", name=f"den_{c}")
            nc.tensor.matmul(den_ps, ones, dacc,
                             start=True, stop=False)
            nc.tensor.matmul(den_ps[:, e_last_lo:], ones,
                             e_last[:, e_last_lo:], start=False, stop=True)
            den_sb = denp.tile([1, 512], F32, tag="den_sb", bufs=1)
            nc.vector.tensor_copy(den_sb, den_ps)
            # transpose [1,512] -> [128,4] on PE: contraction-1 matmuls
            den_t = ps1.tile([128, 4], F32, tag="ps")
            for t in range(4):
                nc.tensor.matmul(den_t[:, t:t + 1],
                                 den_sb[0:1, t * 128:(t + 1) * 128],
                                 ones_f[0:1, 0:1], skip_group_check=True)
            # reciprocal with one Newton step (raw DVE recip is ~2% off),
            # done on the tiny [128,4] per-partition layout
            r0t = denp.tile([128, 4], F32, tag="r0")
            nc.vector.reciprocal(r0t, den_t)
            t1 = denp.tile([128, 4], F32, tag="t1")
            nc.vector.tensor_mul(t1, den_t, r0t)
            nc.vector.tensor_scalar(t1, t1, -1.0, 2.0,
                                    op0=ALU.mult, op1=ALU.add)
            recip_pt = denp.tile([128, 4], F32, tag="recip")
            nc.vector.tensor_mul(recip_pt, r0t, t1)

            attn_cur = [None] * 4
            for j in range(4):
                asb = asbp.tile([128, 512], F32R, tag=f"as{j}",
                                name=f"as{j}_{c}")
                nc.scalar.copy(asb[:, 0:256], attn_ps[j][:, 0:256])
                nc.vector.tensor_copy(asb[:, 256:512], attn_ps[j][:, 256:512])
                attn_cur[j] = asb

            if c + 1 < NCH:
                pend = ln_chunk(c + 1, xc0)

            # ---------------- project + residual for this chunk ------------
            t_order = (3, 0, 1, 2) if c == NCH - 1 else (0, 1, 2, 3)
            for ti, t in enumerate(t_order):
                ob = oc[:, t * D:(t + 1) * D]
                if c < NCH - 1:
                    o1 = psB.tile([128, 512], F32, tag="a0")
                    for j in range(4):
                        nc.tensor.matmul(o1, glt[j][:, t * 128:(t + 1) * 128],
                                         projsb[:, j * 512:(j + 1) * 512],
                                         start=(j == 0), stop=(j == 3))
                    o2 = psB.tile([128, 512], F32, tag="a1")
                else:
                    o2 = psB.tile([128, 512], F32, tag=("a0", "a1")[ti % 2])
                for j in range(4):
                    nc.tensor.matmul(o2,
                                     attn_cur[j][:, t * 128:(t + 1) * 128],
                                     projsb[:, (4 + j) * 512:(5 + j) * 512],
                                     start=(j == 0), stop=(j == 3))
                a2 = outp.tile([128, 512], F32, tag="a2")
                nc.vector.tensor_scalar(a2, o2, recip_pt[:, t:t + 1], None,
                                        op0=ALU.mult)
                if c < NCH - 1:
                    nc.vector.tensor_add(ob, o1, a2)
                    nc.gpsimd.tensor_add(ob, ob, xc[:, t * D:(t + 1) * D])
                else:
                    # o1 + x already accumulated into oc during attention;
                    # DVE (not Pool) keeps the final store chain short
                    nc.vector.tensor_add(ob, ob, a2)
                    nc.sync.dma_start(
                        y[c * CH + t * 128:c * CH + (t + 1) * 128, :], ob)
            if c < NCH - 1:
                nc.sync.dma_start(
                    y[c * CH:(c + 1) * CH, :].rearrange("(t p) d -> p t d",
                                                        p=128),
                    oc.rearrange("p (t d) -> p t d", t=NCH))


_cached = {}


def _build(loop=None):
    import os

    if loop is None:
        loop = int(os.environ.get("ATTN_LOOP", "0"))
    key = ("nc", loop)
    if key in _cached:
        return _cached[key]
    _apply_patches()
    nc = bass.Bass("TRN2", target_bir_lowering=False, debug=False)
    with tile.TileContext(nc) as tc:
        if loop > 1:
            with tc.For_i(0, loop, 1):
                _emit(nc, tc)
        else:
            _emit(nc, tc)
    _cached[key] = nc
    return nc


def _host_prep(expand, project, position_bias_mult):
    E0 = np.array(expand, dtype=np.float32)
    # reorder columns to [q | k | local-lin | local-gelu | v-lin | v-gelu]
    # original: q 0:64 | k 64:128 | lin 128:1152 | gelu 1152:2176
    #   where gated = lin * gelu(gelu_cols); local = gated[:, :512],
    #   v = gated[:, 512:]
    E = np.empty_like(E0)
    E[:, C_Q:C_Q + 64] = E0[:, 0:64] / np.sqrt(np.float32(QK))  # q (folded)
    E[:, C_K:C_K + 64] = E0[:, 64:128]                          # k
    E[:, C_LL:C_LL + 512] = E0[:, 128:640]                      # local lin
    E[:, C_LG:C_LG + 512] = E0[:, 1152:1664]                    # local gelu
    E[:, C_VL:C_VL + 512] = E0[:, 640:1152]                     # v lin
    E[:, C_VG:C_VG + 512] = E0[:, 1664:2176]                    # v gelu
    pbm = np.float64(position_bias_mult)
    # 5 band mask tile patterns (chunk-independent): band t covers k-rows
    # [t*128,(t+1)*128) of the chunk's diagonal block (t=-1: the 128 rows
    # just before it); mask[k', q'] = sigmoid((k - q) + pbm) where k <= q.
    kk = np.arange(128, dtype=np.float64)[:, None]
    qq = np.arange(CH, dtype=np.float64)[None, :]
    tiles = []
    for t in (-1, 0, 1, 2, 3):
        d = (kk + t * 128) - qq          # k - q
        with np.errstate(over="ignore"):
            m = 1.0 / (1.0 + np.exp(-(d + pbm)))
        tiles.append(np.where(d <= 0, m, -10000.0))
    maskdiag = np.concatenate(tiles, axis=1).astype(np.float16)
    P = np.array(project, dtype=np.float32)
    return E, P, maskdiag


def kernel(x, expand, project, position_bias_mult):
    import os

    nc = _build()
    E, P, maskdiag = _host_prep(expand, project, position_bias_mult)
    xs = np.ascontiguousarray(np.array(x, dtype=np.float32))
    in_maps = [
        {"x": xs[b], "expand": E, "project": P, "maskdiag": maskdiag}
        for b in range(B)
    ]
    trace = bool(int(os.environ.get("ATTN_TRACE", "0")))
    res = bass_utils.run_bass_kernel_spmd(
        nc, in_maps, core_ids=list(range(B)), trace=trace)
    _cached["exec_time_ns"] = res.exec_time_ns
    return np.stack([r["y"] for r in res.results], axis=0)
